# revision 58
# baseline (speedup 1.0000x reference)
"""AttentionBlock (GroupNorm -> qkv -> full 4096-token attention -> GroupNorm
-> SwiGLU MLP -> residual) on 8 Trainium2 NeuronCores.

Sharding: core = (batch b = core//2, query-token half h = core%2). Each core
computes k/v over all 4096 tokens of its image and attention rows for its
2048 query tokens (host permutes tokens so queries are always columns
0..2047 -> one static SPMD program). The attention is computed fully
transposed (S^T = k^T q with keys on partitions) so no transposes are needed
anywhere.

All GEMMs run in fp16 with fp32 PSUM accumulation (fp16 streams at the same
1 col/cycle as bf16 on the PE but carries 10 mantissa bits). GroupNorm-1 is
folded into the q/k/v weights on the host (per-image diag scale on the input
channels), so the kernel consumes raw x and the xn phase disappears.
Softmax row-sums are accumulated on the otherwise-idle Pool engine
([key-partition, query] partials; the final 128-way partition reduction and
the U/rowsum normalization happen on the host between the two launches,
which also folds the MLP GroupNorm into the MLP weights). Attention U tiles
are DMA'd to DRAM straight out of PSUM in fp32.
"""
import sys
from contextlib import ExitStack

for _p in ("/opt/trn_rl_repo", "/root/.axon_site/_ro/trn_rl_repo"):
    if _p not in sys.path:
        sys.path.insert(0, _p)

import numpy as np
import ml_dtypes

import concourse.bass as bass
import concourse.tile as tile
from concourse import bacc, mybir, bass_utils

F32 = mybir.dt.float32
F16 = mybir.dt.float16
AF = mybir.ActivationFunctionType
ALU = mybir.AluOpType

P = 128          # partitions
C = 512          # channels
CT = C // P      # 4 channel tiles (== 4 groups: each group is one c-tile)
HW = 4096        # tokens per image
NT = 2048        # query tokens per core
NI = NT // 512   # i-chunks of 512
NJ = HW // P     # 32 j-tiles of 128
B = 4
EPS = 1e-6
SCALE = C ** -0.5


DR = mybir.MatmulPerfMode.DoubleRow
F8 = mybir.dt.float8e4
EXP_OFF = 3.0


def build_launch_a(repeat: int = 1):
    nc = bacc.Bacc("TRN2", target_bir_lowering=False, debug=False, num_devices=8)

    x = nc.dram_tensor("x", [C, HW], F16, kind="ExternalInput").ap()
    wqT = nc.dram_tensor("wqT", [C, C], F16, kind="ExternalInput").ap()
    wkT = nc.dram_tensor("wkT", [C, C], F16, kind="ExternalInput").ap()
    wvT = nc.dram_tensor("wvT", [C, C], F16, kind="ExternalInput").ap()
    qb = nc.dram_tensor("qb", [P, CT], F32, kind="ExternalInput").ap()
    kb = nc.dram_tensor("kb", [P, CT], F32, kind="ExternalInput").ap()

    u_d = nc.dram_tensor("u", [C, NT], F16, kind="ExternalOutput").ap()
    rsp_d = nc.dram_tensor("rsp", [P, NI * 512], F32, kind="ExternalOutput").ap()

    with tile.TileContext(nc) as tc, ExitStack() as ctx:
        const = ctx.enter_context(tc.tile_pool(name="const", bufs=1))

        big = ctx.enter_context(tc.tile_pool(name="big", bufs=1))
        wq_t = big.tile([P, CT, C], F16)
        wk_t = big.tile([P, CT, C], F16)
        wv_t = big.tile([P, CT, C], F16)
        qb_t = const.tile([P, CT], F32)
        kb_t = const.tile([P, CT], F32)


        k_sb = big.tile([P, CT, HW], F16)
        q_sb = big.tile([P, CT, NT], F16)
        vt_sb = big.tile([P, NJ, C], F16)
        rsp_sb = big.tile([P, NI, 512], F32)   # rowsum partials (Pool engine)

        for rep in range(repeat):
            with tc.tile_pool(name=f"xin_{rep}", bufs=1) as pxin, \
                 tc.tile_pool(name=f"pexp_{rep}", bufs=6) as pexp, \
                 tc.tile_pool(name=f"pu_{rep}", bufs=4) as puo, \
                 tc.tile_pool(name=f"psS_{rep}", bufs=3, space="PSUM") as psS, \
                 tc.tile_pool(name=f"psU_{rep}", bufs=5, space="PSUM") as psU:
                x_sb = pxin.tile([P, CT, HW], F16, name=f"x_{rep}")
                # one serial HWDGE queue: emit DMAs in the order compute
                # consumes them (wk+x0 first; wq/wv fetched under the k GEMMs)
                if rep == 0:
                    for ci in range(CT):
                        nc.sync.dma_start(out=wk_t[:, ci, :],
                                          in_=wkT[ci * P:(ci + 1) * P, :])
                        nc.sync.dma_start(
                            out=x_sb[:, ci, 0:512],
                            in_=x[ci * P:(ci + 1) * P, 0:512])
                    nc.sync.dma_start(out=qb_t, in_=qb)
                    nc.sync.dma_start(out=kb_t, in_=kb)
                    for ci in range(CT):
                        nc.sync.dma_start(out=wq_t[:, ci, :],
                                          in_=wqT[ci * P:(ci + 1) * P, :])
                    for ci in range(CT):
                        nc.sync.dma_start(
                            out=x_sb[:, ci, 512:1024],
                            in_=x[ci * P:(ci + 1) * P, 512:1024])
                    for ci in range(CT):
                        nc.sync.dma_start(out=wv_t[:, ci, :],
                                          in_=wvT[ci * P:(ci + 1) * P, :])
                    first_jc = 2
                else:
                    first_jc = 0
                for jc in range(first_jc, HW // 512):
                    for ci in range(CT):
                        nc.sync.dma_start(
                            out=x_sb[:, ci, jc * 512:(jc + 1) * 512],
                            in_=x[ci * P:(ci + 1) * P, jc * 512:(jc + 1) * 512])

                # ---- phase 1: k / q / vT GEMMs straight off x, one pass
                # over the token chunks; vT lags 2 chunks so wv can arrive
                def emit_v(jt):
                    pv = psS.tile([P, C], F32, tag="S")
                    for ci in range(CT):
                        nc.tensor.matmul(pv, x_sb[:, ci, jt * P:(jt + 1) * P],
                                         wv_t[:, ci, :],
                                         start=(ci == 0), stop=(ci == CT - 1))
                    nc.vector.tensor_copy(out=vt_sb[:, jt, :], in_=pv)

                for jc in range(HW // 512):
                    jsl = slice(jc * 512, (jc + 1) * 512)
                    for co in range(CT):
                        pk = psS.tile([P, 512], F32, tag="S")
                        for ci in range(CT):
                            nc.tensor.matmul(pk, wk_t[:, ci, co * P:(co + 1) * P],
                                             x_sb[:, ci, jsl],
                                             start=(ci == 0), stop=(ci == CT - 1))
                        nc.vector.tensor_scalar_add(
                            out=k_sb[:, co, jsl], in0=pk,
                            scalar1=kb_t[:, co:co + 1])
                    if jc < NI:
                        for co in range(CT):
                            pq = psS.tile([P, 512], F32, tag="S")
                            for ci in range(CT):
                                nc.tensor.matmul(pq, wq_t[:, ci, co * P:(co + 1) * P],
                                                 x_sb[:, ci, jsl],
                                                 start=(ci == 0), stop=(ci == CT - 1))
                            nc.vector.tensor_scalar_add(
                                out=q_sb[:, co, jsl], in0=pq,
                                scalar1=qb_t[:, co:co + 1])
                    if jc >= 2:
                        for jt in range(4 * (jc - 2), 4 * (jc - 2) + 4):
                            emit_v(jt)
                for jt in range(NJ - 8, NJ):
                    emit_v(jt)

                # ---- phase 2: attention (S^T -> exp -> U; rowsum on Pool) ----
                for ic in range(NI):
                    isl = slice(ic * 512, (ic + 1) * 512)
                    s_ps = []
                    u_ps = [psU.tile([P, 512], F32, tag="u",
                                     name=f"u_{rep}_{ic}_{cc}")
                            for cc in range(CT)]

                    def emit_S(jt):
                        ps = psS.tile([P, 512], F32, tag="S")
                        for ci in range(CT):
                            nc.tensor.matmul(ps, k_sb[:, ci, jt * P:(jt + 1) * P],
                                             q_sb[:, ci, isl],
                                             start=(ci == 0), stop=(ci == CT - 1))
                        s_ps.append(ps)

                    emit_S(0)
                    emit_S(1)
                    for jt in range(NJ):
                        if jt + 2 < NJ:
                            emit_S(jt + 2)
                        et = pexp.tile([P, 512], F16, tag="e",
                                       name=f"e_{rep}_{ic}_{jt}")
                        nc.scalar.activation(out=et, in_=s_ps[jt],
                                             func=AF.Exp, scale=SCALE)
                        for cc in range(CT):
                            nc.tensor.matmul(u_ps[cc],
                                             vt_sb[:, jt, cc * P:(cc + 1) * P],
                                             et,
                                             start=(jt == 0), stop=(jt == NJ - 1))
                        # rowsum partials on the Pool engine
                        if jt == 0:
                            nc.gpsimd.tensor_copy(out=rsp_sb[:, ic, :], in_=et)
                        else:
                            nc.gpsimd.tensor_tensor(out=rsp_sb[:, ic, :],
                                                    in0=rsp_sb[:, ic, :],
                                                    in1=et, op=ALU.add)

                    # drain U on DVE; last chunk splits DVE/ACT for the tail
                    last = ic == NI - 1
                    for cc in range(CT):
                        ut = puo.tile([P, 512], F16, tag="uo")
                        if last and cc % 2 == 1:
                            nc.scalar.activation(out=ut, in_=u_ps[cc],
                                                 func=AF.Copy, scale=1.0)
                        else:
                            nc.vector.tensor_copy(out=ut, in_=u_ps[cc])
                        nc.sync.dma_start(out=u_d[cc * P:(cc + 1) * P, isl],
                                          in_=ut)
                    nc.sync.dma_start(out=rsp_d[:, isl], in_=rsp_sb[:, ic, :])

    nc.compile()
    return nc


def build_launch_b(repeat: int = 1):
    nc = bacc.Bacc("TRN2", target_bir_lowering=False, debug=False, num_devices=8)

    on = nc.dram_tensor("on", [C, NT], F16, kind="ExternalInput").ap()
    w1T = nc.dram_tensor("w1T", [C, 2 * C], F16, kind="ExternalInput").ap()
    c1 = nc.dram_tensor("c1", [P, 2 * CT], F32, kind="ExternalInput").ap()
    w2T = nc.dram_tensor("w2T", [C, C], F16, kind="ExternalInput").ap()
    b2 = nc.dram_tensor("b2", [P, CT], F32, kind="ExternalInput").ap()

    y = nc.dram_tensor("y", [C, NT], F16, kind="ExternalOutput").ap()

    with tile.TileContext(nc) as tc, ExitStack() as ctx:
        big = ctx.enter_context(tc.tile_pool(name="big", bufs=1))
        psG = ctx.enter_context(tc.tile_pool(name="psG", bufs=4, space="PSUM"))
        pout = ctx.enter_context(tc.tile_pool(name="pout", bufs=4))

        on_t = big.tile([P, CT, NT], F16)
        w1_t = big.tile([P, CT, 2 * C], F16)
        w2_t = big.tile([P, CT, C], F16)
        c1_t = big.tile([P, 2 * CT], F32)
        b2_t = big.tile([P, CT], F32)
        # prefetch in the order chunk-0 compute consumes it: w1 g-half +
        # on chunk0, then w1 a-half, then w2, then the remaining on chunks
        for ci in range(CT):
            nc.sync.dma_start(out=w1_t[:, ci, C:], in_=w1T[ci * P:(ci + 1) * P, C:])
            nc.sync.dma_start(out=on_t[:, ci, 0:512], in_=on[ci * P:(ci + 1) * P, 0:512])
        nc.sync.dma_start(out=c1_t, in_=c1)
        nc.sync.dma_start(out=b2_t, in_=b2)
        for ci in range(CT):
            nc.sync.dma_start(out=w1_t[:, ci, :C], in_=w1T[ci * P:(ci + 1) * P, :C])
        for ci in range(CT):
            nc.sync.dma_start(out=w2_t[:, ci, :], in_=w2T[ci * P:(ci + 1) * P, :])
        for chk in range(1, NI):
            for ci in range(CT):
                nc.sync.dma_start(out=on_t[:, ci, chk * 512:(chk + 1) * 512],
                                  in_=on[ci * P:(ci + 1) * P, chk * 512:(chk + 1) * 512])

        z_sb = big.tile([P, CT, NT], F32)    # (g + c1g) * sigmoid(g + c1g)
        s_sb = big.tile([P, CT, NT], F32)    # sigmoid(g + c1g)
        h_sb = big.tile([P, CT, NT], F16)    # a * silu

        for ic in range(NI * repeat):
            ic = ic % NI
            isl = slice(ic * 512, (ic + 1) * 512)
            # 'g' half first: zs = (g + c1g) * sigmoid(g + c1g)
            for gt in range(CT):
                pm = psG.tile([P, 512], F32, tag="m1")
                for ci in range(CT):
                    nc.tensor.matmul(pm, w1_t[:, ci, (CT + gt) * P:(CT + gt + 1) * P],
                                     on_t[:, ci, isl],
                                     start=(ci == 0), stop=(ci == CT - 1))
                nc.scalar.activation(out=s_sb[:, gt, isl], in_=pm,
                                     func=AF.Sigmoid,
                                     bias=c1_t[:, CT + gt:CT + gt + 1], scale=1.0)
                nc.vector.scalar_tensor_tensor(out=z_sb[:, gt, isl], in0=pm,
                                               scalar=c1_t[:, CT + gt:CT + gt + 1],
                                               in1=s_sb[:, gt, isl],
                                               op0=ALU.add, op1=ALU.mult)
            # 'a' half: h = (a + c1a) * zs, fused from PSUM
            for ot in range(CT):
                pm = psG.tile([P, 512], F32, tag="m1")
                for ci in range(CT):
                    nc.tensor.matmul(pm, w1_t[:, ci, ot * P:(ot + 1) * P],
                                     on_t[:, ci, isl],
                                     start=(ci == 0), stop=(ci == CT - 1))
                nc.vector.scalar_tensor_tensor(out=h_sb[:, ot, isl], in0=pm,
                                               scalar=c1_t[:, ot:ot + 1],
                                               in1=z_sb[:, ot, isl],
                                               op0=ALU.add, op1=ALU.mult)
            for ot in range(CT):
                pm2 = psG.tile([P, 512], F32, tag="m2")
                for cc in range(CT):
                    nc.tensor.matmul(pm2, w2_t[:, cc, ot * P:(ot + 1) * P],
                                     h_sb[:, cc, isl],
                                     start=(cc == 0), stop=(cc == CT - 1))
                yt = pout.tile([P, 512], F16, tag="yt")
                if ot % 2 == 0:
                    nc.vector.tensor_scalar_add(out=yt, in0=pm2,
                                                scalar1=b2_t[:, ot:ot + 1])
                else:
                    nc.scalar.activation(out=yt, in_=pm2, func=AF.Identity,
                                         bias=b2_t[:, ot:ot + 1], scale=1.0)
                nc.sync.dma_start(out=y[ot * P:(ot + 1) * P, isl], in_=yt)

    nc.compile()
    return nc


def _tile_vec(v):
    """[C] -> [P, CT] with partition = channel % 128, col = channel // 128."""
    return np.ascontiguousarray(np.asarray(v, np.float32).reshape(-1, P).T)


_CACHE = {}


def _get_ncs():
    if "a" not in _CACHE:
        _CACHE["a"] = build_launch_a()
        _CACHE["b"] = build_launch_b()
    return _CACHE["a"], _CACHE["b"]


def prep_a_inmaps(inputs):
    x = np.asarray(inputs["x"], np.float32).reshape(B, C, HW)
    wq = np.asarray(inputs["q_w"], np.float64)
    wk = np.asarray(inputs["k_w"], np.float64)
    wv = np.asarray(inputs["v_w"], np.float64)
    nsc = np.asarray(inputs["norm_scale"], np.float64)
    nbi = np.asarray(inputs["norm_bias"], np.float64)
    qb0 = np.asarray(inputs["q_b"], np.float64)
    kb0 = np.asarray(inputs["k_b"], np.float64)

    a_maps = []
    per_image = []
    for b in range(B):
        # GroupNorm-1 stats on host (f64), folded into the qkv weights
        g = x[b].reshape(CT, P * HW).astype(np.float64)
        mean_g = g.mean(axis=1)
        var_g = g.var(axis=1)
        rstd_c = np.repeat(1.0 / np.sqrt(var_g + EPS), P)
        mean_c = np.repeat(mean_g, P)
        alpha = rstd_c * nsc
        beta = nbi - mean_c * rstd_c * nsc
        wqT = np.ascontiguousarray((wq * alpha[None, :]).T).astype(np.float16)
        wkT = np.ascontiguousarray((wk * alpha[None, :]).T).astype(np.float16)
        wvT = np.ascontiguousarray((wv * alpha[None, :]).T).astype(np.float16)
        qbe = _tile_vec(qb0 + wq @ beta)
        kbe = _tile_vec(kb0 + wk @ beta)
        per_image.append((wqT, wkT, wvT, qbe, kbe))

    for core in range(8):
        b, h = core // 2, core % 2
        xb = x[b]
        xp = xb if h == 0 else np.ascontiguousarray(
            np.concatenate([xb[:, NT:], xb[:, :NT]], axis=1))
        wqT, wkT, wvT, qbe, kbe = per_image[b]
        a_maps.append(dict(x=xp.astype(np.float16), wqT=wqT, wkT=wkT, wvT=wvT,
                           qb=qbe, kb=kbe))
    return a_maps


def normalize_a_results(inputs, results):
    # ---- host: normalize softmax, add (folded) v bias, GroupNorm-2 stats ----
    x = np.asarray(inputs["x"], np.float32).reshape(B, C, HW)
    wv = np.asarray(inputs["v_w"], np.float64)
    nsc = np.asarray(inputs["norm_scale"], np.float64)
    nbi = np.asarray(inputs["norm_bias"], np.float64)
    vb0 = np.asarray(inputs["v_b"], np.float64)
    vb_eff = []
    for b in range(B):
        g = x[b].reshape(CT, P * HW).astype(np.float64)
        mean_g = g.mean(axis=1)
        var_g = g.var(axis=1)
        rstd_c = np.repeat(1.0 / np.sqrt(var_g + EPS), P)
        mean_c = np.repeat(mean_g, P)
        beta = nbi - mean_c * rstd_c * nsc
        vb_eff.append((vb0 + wv @ beta).astype(np.float32))

    norm = []
    for core, r in enumerate(results):
        b = core // 2
        U = r["u"].astype(np.float32)
        rs = r["rsp"].astype(np.float32).sum(axis=0)       # [NT]
        out = U / rs[None, :] + vb_eff[b][:, None]
        outh = out.astype(np.float16)
        of = outh.astype(np.float64).reshape(CT, P, NT)
        pst = np.empty((P, 2 * CT), np.float64)
        pst[:, 0::2] = of.sum(axis=2).T
        pst[:, 1::2] = (of ** 2).sum(axis=2).T
        norm.append(dict(out_n=outh, pstats=pst))
    return norm


def combine_stats_and_prep_b(inputs, norm):
    w1 = np.asarray(inputs["mlp_w1"], np.float32)
    b1 = np.asarray(inputs["mlp_b1"], np.float32)
    w2 = np.asarray(inputs["mlp_w2"], np.float32)
    msc = np.asarray(inputs["mlp_norm_scale"], np.float32)
    mbi = np.asarray(inputs["mlp_norm_bias"], np.float32)
    w2T = np.ascontiguousarray(w2.T).astype(np.float16)
    b2t = _tile_vec(inputs["mlp_b2"])

    b_maps = []
    for core in range(8):
        b, h = core // 2, core % 2
        ps = norm[2 * b]["pstats"] + norm[2 * b + 1]["pstats"]
        S = ps[:, 0::2].sum(axis=0)
        SQ = ps[:, 1::2].sum(axis=0)
        N = P * HW
        mean_g = S / N
        var_g = SQ / N - mean_g ** 2
        rstd_g = 1.0 / np.sqrt(var_g + EPS)
        mean_c = np.repeat(mean_g, P)
        rstd_c = np.repeat(rstd_g, P)
        alpha2 = (rstd_c * msc).astype(np.float32)
        beta2 = (mbi - mean_c * rstd_c * msc).astype(np.float32)
        w1Ts = np.ascontiguousarray(w1.T * alpha2[:, None]).astype(np.float16)
        c1 = (b1 + w1 @ beta2).astype(np.float32)
        c1t = np.ascontiguousarray(c1.reshape(2 * CT, P).T)
        b_maps.append(dict(on=norm[core]["out_n"], w1T=w1Ts,
                           c1=c1t, w2T=w2T, b2=b2t))
    return b_maps


def assemble_y(inputs, results):
    x = np.asarray(inputs["x"], np.float32).reshape(B, C, HW)
    y = np.empty((B, C, HW), np.float32)
    for core in range(8):
        b, h = core // 2, core % 2
        y[b][:, h * NT:(h + 1) * NT] = (results[core]["y"].astype(np.float32)
                                        + x[b][:, h * NT:(h + 1) * NT])
    return y.reshape(B, C, 64, 64)


def kernel(**inputs):
    nca, ncb = _get_ncs()
    a_maps = prep_a_inmaps(inputs)
    res_a = bass_utils.run_bass_kernel_spmd(nca, a_maps, core_ids=list(range(8)))
    norm = normalize_a_results(inputs, res_a.results)
    b_maps = combine_stats_and_prep_b(inputs, norm)
    res_b = bass_utils.run_bass_kernel_spmd(ncb, b_maps, core_ids=list(range(8)))
    return assemble_y(inputs, res_b.results)


# revision 59
# speedup vs baseline: 1.0486x; 1.0486x over previous
"""AttentionBlock (GroupNorm -> qkv -> full 4096-token attention -> GroupNorm
-> SwiGLU MLP -> residual) on 8 Trainium2 NeuronCores.

Sharding: core = (batch b = core//2, query-token half h = core%2). Each core
computes k/v over all 4096 tokens of its image and attention rows for its
2048 query tokens (host permutes tokens so queries are always columns
0..2047 -> one static SPMD program). The attention is computed fully
transposed (S^T = k^T q with keys on partitions) so no transposes are needed
anywhere.

All GEMMs run in fp16 with fp32 PSUM accumulation (fp16 streams at the same
1 col/cycle as bf16 on the PE but carries 10 mantissa bits). GroupNorm-1 is
folded into the q/k/v weights on the host (per-image diag scale on the input
channels), so the kernel consumes raw x and the xn phase disappears.
Softmax row-sums are accumulated on the otherwise-idle Pool engine
([key-partition, query] partials; the final 128-way partition reduction and
the U/rowsum normalization happen on the host between the two launches,
which also folds the MLP GroupNorm into the MLP weights). Attention U tiles
are DMA'd to DRAM straight out of PSUM in fp32.
"""
import sys
from contextlib import ExitStack

for _p in ("/opt/trn_rl_repo", "/root/.axon_site/_ro/trn_rl_repo"):
    if _p not in sys.path:
        sys.path.insert(0, _p)

import numpy as np
import ml_dtypes

import concourse.bass as bass
import concourse.tile as tile
from concourse import bacc, mybir, bass_utils

F32 = mybir.dt.float32
F16 = mybir.dt.float16
AF = mybir.ActivationFunctionType
ALU = mybir.AluOpType

P = 128          # partitions
C = 512          # channels
CT = C // P      # 4 channel tiles (== 4 groups: each group is one c-tile)
HW = 4096        # tokens per image
NT = 2048        # query tokens per core
NI = NT // 512   # i-chunks of 512
NJ = HW // P     # 32 j-tiles of 128
B = 4
EPS = 1e-6
SCALE = C ** -0.5


DR = mybir.MatmulPerfMode.DoubleRow
F8 = mybir.dt.float8e4
EXP_OFF = 3.0


def build_launch_a(repeat: int = 1):
    nc = bacc.Bacc("TRN2", target_bir_lowering=False, debug=False, num_devices=8)

    x = nc.dram_tensor("x", [C, NT], F16, kind="ExternalInput").ap()
    wqT = nc.dram_tensor("wqT", [C, C], F16, kind="ExternalInput").ap()
    wkT = nc.dram_tensor("wkT", [C, C], F16, kind="ExternalInput").ap()
    wvT = nc.dram_tensor("wvT", [C, C], F16, kind="ExternalInput").ap()
    qb = nc.dram_tensor("qb", [P, CT], F32, kind="ExternalInput").ap()
    kb = nc.dram_tensor("kb", [P, CT], F32, kind="ExternalInput").ap()

    u_d = nc.dram_tensor("u", [C, NT], F16, kind="ExternalOutput").ap()
    rsp_d = nc.dram_tensor("rsp", [P, NI * 512], F32, kind="ExternalOutput").ap()

    # k/v pair-exchange: each core computes k/v for its own 2048 tokens; an
    # AllGather within the core pair assembles all 4096 in rank-major key
    # order (fine - attention reduces over keys, any consistent order works)
    PAIRS = [[2 * i, 2 * i + 1] for i in range(4)]
    kx_in = nc.dram_tensor("kx_in", [C, NT], F16).ap()
    kx_out = nc.dram_tensor("kx_out", [2, C, NT], F16).ap()
    vx_in = nc.dram_tensor("vx_in", [P, 16 * C], F16).ap()
    vx_out = nc.dram_tensor("vx_out", [2, P, 16 * C], F16).ap()

    with tile.TileContext(nc) as tc, ExitStack() as ctx:
        const = ctx.enter_context(tc.tile_pool(name="const", bufs=1))

        big = ctx.enter_context(tc.tile_pool(name="big", bufs=1))
        wq_t = big.tile([P, CT, C], F16)
        wk_t = big.tile([P, CT, C], F16)
        wv_t = big.tile([P, CT, C], F16)
        qb_t = const.tile([P, CT], F32)
        kb_t = const.tile([P, CT], F32)


        k_sb = big.tile([P, CT, HW], F16)
        q_sb = big.tile([P, CT, NT], F16)
        vt_sb = big.tile([P, NJ, C], F16)
        rsp_sb = big.tile([P, NI, 512], F32)   # rowsum partials (Pool engine)

        for rep in range(repeat):
            with tc.tile_pool(name=f"xin_{rep}", bufs=1) as pxin, \
                 tc.tile_pool(name=f"pexp_{rep}", bufs=6) as pexp, \
                 tc.tile_pool(name=f"pu_{rep}", bufs=4) as puo, \
                 tc.tile_pool(name=f"psS_{rep}", bufs=3, space="PSUM") as psS, \
                 tc.tile_pool(name=f"psU_{rep}", bufs=5, space="PSUM") as psU:
                x_sb = pxin.tile([P, CT, NT], F16, name=f"x_{rep}")
                # one serial HWDGE queue: emit DMAs in the order compute
                # consumes them (wk+x0 first; wq/wv fetched under the k GEMMs)
                if rep == 0:
                    for ci in range(CT):
                        nc.sync.dma_start(out=wk_t[:, ci, :],
                                          in_=wkT[ci * P:(ci + 1) * P, :])
                        nc.sync.dma_start(
                            out=x_sb[:, ci, 0:512],
                            in_=x[ci * P:(ci + 1) * P, 0:512])
                    nc.sync.dma_start(out=qb_t, in_=qb)
                    nc.sync.dma_start(out=kb_t, in_=kb)
                    for ci in range(CT):
                        nc.sync.dma_start(out=wq_t[:, ci, :],
                                          in_=wqT[ci * P:(ci + 1) * P, :])
                    for ci in range(CT):
                        nc.sync.dma_start(
                            out=x_sb[:, ci, 512:1024],
                            in_=x[ci * P:(ci + 1) * P, 512:1024])
                    for ci in range(CT):
                        nc.sync.dma_start(out=wv_t[:, ci, :],
                                          in_=wvT[ci * P:(ci + 1) * P, :])
                    first_jc = 2
                else:
                    first_jc = 0
                for jc in range(first_jc, NI):
                    for ci in range(CT):
                        nc.sync.dma_start(
                            out=x_sb[:, ci, jc * 512:(jc + 1) * 512],
                            in_=x[ci * P:(ci + 1) * P, jc * 512:(jc + 1) * 512])

                # ---- phase 1: k / q / vT GEMMs straight off x, one pass
                # over the token chunks; vT lags 2 chunks so wv can arrive
                def emit_v(jt):
                    pv = psS.tile([P, C], F32, tag="S")
                    for ci in range(CT):
                        nc.tensor.matmul(pv, x_sb[:, ci, jt * P:(jt + 1) * P],
                                         wv_t[:, ci, :],
                                         start=(ci == 0), stop=(ci == CT - 1))
                    nc.vector.tensor_copy(out=vt_sb[:, jt, :], in_=pv)

                for jc in range(NI):
                    jsl = slice(jc * 512, (jc + 1) * 512)
                    for co in range(CT):
                        pk = psS.tile([P, 512], F32, tag="S")
                        for ci in range(CT):
                            nc.tensor.matmul(pk, wk_t[:, ci, co * P:(co + 1) * P],
                                             x_sb[:, ci, jsl],
                                             start=(ci == 0), stop=(ci == CT - 1))
                        nc.vector.tensor_scalar_add(
                            out=k_sb[:, co, jsl], in0=pk,
                            scalar1=kb_t[:, co:co + 1])
                    for co in range(CT):
                        pq = psS.tile([P, 512], F32, tag="S")
                        for ci in range(CT):
                            nc.tensor.matmul(pq, wq_t[:, ci, co * P:(co + 1) * P],
                                             x_sb[:, ci, jsl],
                                             start=(ci == 0), stop=(ci == CT - 1))
                        nc.vector.tensor_scalar_add(
                            out=q_sb[:, co, jsl], in0=pq,
                            scalar1=qb_t[:, co:co + 1])
                # k exchange: own half out, AllGather in the pair, both back
                for ci in range(CT):
                    nc.sync.dma_start(out=kx_in[ci * P:(ci + 1) * P, :],
                                      in_=k_sb[:, ci, 0:NT])
                nc.gpsimd.collective_compute(
                    "AllGather", ALU.bypass, replica_groups=PAIRS,
                    ins=[kx_in], outs=[kx_out])
                for sl in range(2):
                    for ci in range(CT):
                        nc.sync.dma_start(
                            out=k_sb[:, ci, sl * NT:(sl + 1) * NT],
                            in_=kx_out[sl, ci * P:(ci + 1) * P, :])
                for jt in range(16):
                    emit_v(jt)
                nc.sync.dma_start(out=vx_in, in_=vt_sb[:, 0:16, :])
                nc.gpsimd.collective_compute(
                    "AllGather", ALU.bypass, replica_groups=PAIRS,
                    ins=[vx_in], outs=[vx_out])
                for sl in range(2):
                    nc.sync.dma_start(out=vt_sb[:, sl * 16:(sl + 1) * 16, :],
                                      in_=vx_out[sl])

                # ---- phase 2: attention (S^T -> exp -> U; rowsum on Pool) ----
                for ic in range(NI):
                    isl = slice(ic * 512, (ic + 1) * 512)
                    s_ps = []
                    u_ps = [psU.tile([P, 512], F32, tag="u",
                                     name=f"u_{rep}_{ic}_{cc}")
                            for cc in range(CT)]

                    def emit_S(jt):
                        ps = psS.tile([P, 512], F32, tag="S")
                        for ci in range(CT):
                            nc.tensor.matmul(ps, k_sb[:, ci, jt * P:(jt + 1) * P],
                                             q_sb[:, ci, isl],
                                             start=(ci == 0), stop=(ci == CT - 1))
                        s_ps.append(ps)

                    emit_S(0)
                    emit_S(1)
                    for jt in range(NJ):
                        if jt + 2 < NJ:
                            emit_S(jt + 2)
                        et = pexp.tile([P, 512], F16, tag="e",
                                       name=f"e_{rep}_{ic}_{jt}")
                        nc.scalar.activation(out=et, in_=s_ps[jt],
                                             func=AF.Exp, scale=SCALE)
                        for cc in range(CT):
                            nc.tensor.matmul(u_ps[cc],
                                             vt_sb[:, jt, cc * P:(cc + 1) * P],
                                             et,
                                             start=(jt == 0), stop=(jt == NJ - 1))
                        # rowsum partials on the Pool engine
                        if jt == 0:
                            nc.gpsimd.tensor_copy(out=rsp_sb[:, ic, :], in_=et)
                        else:
                            nc.gpsimd.tensor_tensor(out=rsp_sb[:, ic, :],
                                                    in0=rsp_sb[:, ic, :],
                                                    in1=et, op=ALU.add)

                    # drain U on DVE; last chunk splits DVE/ACT for the tail
                    last = ic == NI - 1
                    for cc in range(CT):
                        ut = puo.tile([P, 512], F16, tag="uo")
                        if last and cc % 2 == 1:
                            nc.scalar.activation(out=ut, in_=u_ps[cc],
                                                 func=AF.Copy, scale=1.0)
                        else:
                            nc.vector.tensor_copy(out=ut, in_=u_ps[cc])
                        nc.sync.dma_start(out=u_d[cc * P:(cc + 1) * P, isl],
                                          in_=ut)
                    nc.sync.dma_start(out=rsp_d[:, isl], in_=rsp_sb[:, ic, :])

    nc.compile()
    return nc


def build_launch_b(repeat: int = 1):
    nc = bacc.Bacc("TRN2", target_bir_lowering=False, debug=False, num_devices=8)

    on = nc.dram_tensor("on", [C, NT], F16, kind="ExternalInput").ap()
    w1T = nc.dram_tensor("w1T", [C, 2 * C], F16, kind="ExternalInput").ap()
    c1 = nc.dram_tensor("c1", [P, 2 * CT], F32, kind="ExternalInput").ap()
    w2T = nc.dram_tensor("w2T", [C, C], F16, kind="ExternalInput").ap()
    b2 = nc.dram_tensor("b2", [P, CT], F32, kind="ExternalInput").ap()

    y = nc.dram_tensor("y", [C, NT], F16, kind="ExternalOutput").ap()

    with tile.TileContext(nc) as tc, ExitStack() as ctx:
        big = ctx.enter_context(tc.tile_pool(name="big", bufs=1))
        psG = ctx.enter_context(tc.tile_pool(name="psG", bufs=4, space="PSUM"))
        pout = ctx.enter_context(tc.tile_pool(name="pout", bufs=4))

        on_t = big.tile([P, CT, NT], F16)
        w1_t = big.tile([P, CT, 2 * C], F16)
        w2_t = big.tile([P, CT, C], F16)
        c1_t = big.tile([P, 2 * CT], F32)
        b2_t = big.tile([P, CT], F32)
        # prefetch in the order chunk-0 compute consumes it: w1 g-half +
        # on chunk0, then w1 a-half, then w2, then the remaining on chunks
        for ci in range(CT):
            nc.sync.dma_start(out=w1_t[:, ci, C:], in_=w1T[ci * P:(ci + 1) * P, C:])
            nc.sync.dma_start(out=on_t[:, ci, 0:512], in_=on[ci * P:(ci + 1) * P, 0:512])
        nc.sync.dma_start(out=c1_t, in_=c1)
        nc.sync.dma_start(out=b2_t, in_=b2)
        for ci in range(CT):
            nc.sync.dma_start(out=w1_t[:, ci, :C], in_=w1T[ci * P:(ci + 1) * P, :C])
        for ci in range(CT):
            nc.sync.dma_start(out=w2_t[:, ci, :], in_=w2T[ci * P:(ci + 1) * P, :])
        for chk in range(1, NI):
            for ci in range(CT):
                nc.sync.dma_start(out=on_t[:, ci, chk * 512:(chk + 1) * 512],
                                  in_=on[ci * P:(ci + 1) * P, chk * 512:(chk + 1) * 512])

        z_sb = big.tile([P, CT, NT], F32)    # (g + c1g) * sigmoid(g + c1g)
        s_sb = big.tile([P, CT, NT], F32)    # sigmoid(g + c1g)
        h_sb = big.tile([P, CT, NT], F16)    # a * silu

        for ic in range(NI * repeat):
            ic = ic % NI
            isl = slice(ic * 512, (ic + 1) * 512)
            # 'g' half first: zs = (g + c1g) * sigmoid(g + c1g)
            for gt in range(CT):
                pm = psG.tile([P, 512], F32, tag="m1")
                for ci in range(CT):
                    nc.tensor.matmul(pm, w1_t[:, ci, (CT + gt) * P:(CT + gt + 1) * P],
                                     on_t[:, ci, isl],
                                     start=(ci == 0), stop=(ci == CT - 1))
                nc.scalar.activation(out=s_sb[:, gt, isl], in_=pm,
                                     func=AF.Sigmoid,
                                     bias=c1_t[:, CT + gt:CT + gt + 1], scale=1.0)
                nc.vector.scalar_tensor_tensor(out=z_sb[:, gt, isl], in0=pm,
                                               scalar=c1_t[:, CT + gt:CT + gt + 1],
                                               in1=s_sb[:, gt, isl],
                                               op0=ALU.add, op1=ALU.mult)
            # 'a' half: h = (a + c1a) * zs, fused from PSUM
            for ot in range(CT):
                pm = psG.tile([P, 512], F32, tag="m1")
                for ci in range(CT):
                    nc.tensor.matmul(pm, w1_t[:, ci, ot * P:(ot + 1) * P],
                                     on_t[:, ci, isl],
                                     start=(ci == 0), stop=(ci == CT - 1))
                nc.vector.scalar_tensor_tensor(out=h_sb[:, ot, isl], in0=pm,
                                               scalar=c1_t[:, ot:ot + 1],
                                               in1=z_sb[:, ot, isl],
                                               op0=ALU.add, op1=ALU.mult)
            for ot in range(CT):
                pm2 = psG.tile([P, 512], F32, tag="m2")
                for cc in range(CT):
                    nc.tensor.matmul(pm2, w2_t[:, cc, ot * P:(ot + 1) * P],
                                     h_sb[:, cc, isl],
                                     start=(cc == 0), stop=(cc == CT - 1))
                yt = pout.tile([P, 512], F16, tag="yt")
                if ot % 2 == 0:
                    nc.vector.tensor_scalar_add(out=yt, in0=pm2,
                                                scalar1=b2_t[:, ot:ot + 1])
                else:
                    nc.scalar.activation(out=yt, in_=pm2, func=AF.Identity,
                                         bias=b2_t[:, ot:ot + 1], scale=1.0)
                nc.sync.dma_start(out=y[ot * P:(ot + 1) * P, isl], in_=yt)

    nc.compile()
    return nc


def _tile_vec(v):
    """[C] -> [P, CT] with partition = channel % 128, col = channel // 128."""
    return np.ascontiguousarray(np.asarray(v, np.float32).reshape(-1, P).T)


_CACHE = {}


def _get_ncs():
    if "a" not in _CACHE:
        _CACHE["a"] = build_launch_a()
        _CACHE["b"] = build_launch_b()
    return _CACHE["a"], _CACHE["b"]


def prep_a_inmaps(inputs):
    x = np.asarray(inputs["x"], np.float32).reshape(B, C, HW)
    wq = np.asarray(inputs["q_w"], np.float64)
    wk = np.asarray(inputs["k_w"], np.float64)
    wv = np.asarray(inputs["v_w"], np.float64)
    nsc = np.asarray(inputs["norm_scale"], np.float64)
    nbi = np.asarray(inputs["norm_bias"], np.float64)
    qb0 = np.asarray(inputs["q_b"], np.float64)
    kb0 = np.asarray(inputs["k_b"], np.float64)

    a_maps = []
    per_image = []
    for b in range(B):
        # GroupNorm-1 stats on host (f64), folded into the qkv weights
        g = x[b].reshape(CT, P * HW).astype(np.float64)
        mean_g = g.mean(axis=1)
        var_g = g.var(axis=1)
        rstd_c = np.repeat(1.0 / np.sqrt(var_g + EPS), P)
        mean_c = np.repeat(mean_g, P)
        alpha = rstd_c * nsc
        beta = nbi - mean_c * rstd_c * nsc
        wqT = np.ascontiguousarray((wq * alpha[None, :]).T).astype(np.float16)
        wkT = np.ascontiguousarray((wk * alpha[None, :]).T).astype(np.float16)
        wvT = np.ascontiguousarray((wv * alpha[None, :]).T).astype(np.float16)
        qbe = _tile_vec(qb0 + wq @ beta)
        kbe = _tile_vec(kb0 + wk @ beta)
        per_image.append((wqT, wkT, wvT, qbe, kbe))

    for core in range(8):
        b, h = core // 2, core % 2
        xp = np.ascontiguousarray(x[b][:, h * NT:(h + 1) * NT])
        wqT, wkT, wvT, qbe, kbe = per_image[b]
        a_maps.append(dict(x=xp.astype(np.float16), wqT=wqT, wkT=wkT, wvT=wvT,
                           qb=qbe, kb=kbe))
    return a_maps


def normalize_a_results(inputs, results):
    # ---- host: normalize softmax, add (folded) v bias, GroupNorm-2 stats ----
    x = np.asarray(inputs["x"], np.float32).reshape(B, C, HW)
    wv = np.asarray(inputs["v_w"], np.float64)
    nsc = np.asarray(inputs["norm_scale"], np.float64)
    nbi = np.asarray(inputs["norm_bias"], np.float64)
    vb0 = np.asarray(inputs["v_b"], np.float64)
    vb_eff = []
    for b in range(B):
        g = x[b].reshape(CT, P * HW).astype(np.float64)
        mean_g = g.mean(axis=1)
        var_g = g.var(axis=1)
        rstd_c = np.repeat(1.0 / np.sqrt(var_g + EPS), P)
        mean_c = np.repeat(mean_g, P)
        beta = nbi - mean_c * rstd_c * nsc
        vb_eff.append((vb0 + wv @ beta).astype(np.float32))

    norm = []
    for core, r in enumerate(results):
        b = core // 2
        U = r["u"].astype(np.float32)
        rs = r["rsp"].astype(np.float32).sum(axis=0)       # [NT]
        out = U / rs[None, :] + vb_eff[b][:, None]
        outh = out.astype(np.float16)
        of = outh.astype(np.float64).reshape(CT, P, NT)
        pst = np.empty((P, 2 * CT), np.float64)
        pst[:, 0::2] = of.sum(axis=2).T
        pst[:, 1::2] = (of ** 2).sum(axis=2).T
        norm.append(dict(out_n=outh, pstats=pst))
    return norm


def combine_stats_and_prep_b(inputs, norm):
    w1 = np.asarray(inputs["mlp_w1"], np.float32)
    b1 = np.asarray(inputs["mlp_b1"], np.float32)
    w2 = np.asarray(inputs["mlp_w2"], np.float32)
    msc = np.asarray(inputs["mlp_norm_scale"], np.float32)
    mbi = np.asarray(inputs["mlp_norm_bias"], np.float32)
    w2T = np.ascontiguousarray(w2.T).astype(np.float16)
    b2t = _tile_vec(inputs["mlp_b2"])

    b_maps = []
    for core in range(8):
        b, h = core // 2, core % 2
        ps = norm[2 * b]["pstats"] + norm[2 * b + 1]["pstats"]
        S = ps[:, 0::2].sum(axis=0)
        SQ = ps[:, 1::2].sum(axis=0)
        N = P * HW
        mean_g = S / N
        var_g = SQ / N - mean_g ** 2
        rstd_g = 1.0 / np.sqrt(var_g + EPS)
        mean_c = np.repeat(mean_g, P)
        rstd_c = np.repeat(rstd_g, P)
        alpha2 = (rstd_c * msc).astype(np.float32)
        beta2 = (mbi - mean_c * rstd_c * msc).astype(np.float32)
        w1Ts = np.ascontiguousarray(w1.T * alpha2[:, None]).astype(np.float16)
        c1 = (b1 + w1 @ beta2).astype(np.float32)
        c1t = np.ascontiguousarray(c1.reshape(2 * CT, P).T)
        b_maps.append(dict(on=norm[core]["out_n"], w1T=w1Ts,
                           c1=c1t, w2T=w2T, b2=b2t))
    return b_maps


def assemble_y(inputs, results):
    x = np.asarray(inputs["x"], np.float32).reshape(B, C, HW)
    y = np.empty((B, C, HW), np.float32)
    for core in range(8):
        b, h = core // 2, core % 2
        y[b][:, h * NT:(h + 1) * NT] = (results[core]["y"].astype(np.float32)
                                        + x[b][:, h * NT:(h + 1) * NT])
    return y.reshape(B, C, 64, 64)


def kernel(**inputs):
    nca, ncb = _get_ncs()
    a_maps = prep_a_inmaps(inputs)
    res_a = bass_utils.run_bass_kernel_spmd(nca, a_maps, core_ids=list(range(8)))
    norm = normalize_a_results(inputs, res_a.results)
    b_maps = combine_stats_and_prep_b(inputs, norm)
    res_b = bass_utils.run_bass_kernel_spmd(ncb, b_maps, core_ids=list(range(8)))
    return assemble_y(inputs, res_b.results)
